# revision 92
# baseline (speedup 1.0000x reference)
"""Baichuan sliding-window GQA attention block on 8 trn2 NeuronCores.

Sharding: data-parallel over batch (2) x tensor-parallel over heads (4).
Core c handles batch b=c//4, head group g=c%4 (q heads 4g..4g+3, kv heads
2g..2g+1). Each core computes qkv projection, RoPE, 2-tap causal conv,
windowed attention and a row-sharded o_proj partial; the host sums the 4
partials per batch.

Single fused pipeline per 256-token chunk: qkv matmuls -> rope/conv ->
banded attention for the chunk -> o_proj partial -> DMA out. All phases
share one in-order instruction stream per engine so PE never waits on a
phase barrier.

Key layout/efficiency choices:
 - weights/activations stream in bf16 (halves DMA), accumulation in fp32.
 - dram tensors are host-pre-arranged to [128, k, cols] so each chunk is
   ONE descriptor-friendly DMA (HWDGE fixed cost is ~625ns per DMA); wpk
   is additionally grouped by the four t=0 col-pair passes so compute
   starts after the first 1MB group lands.
 - on-chip tensors keep a (feature, token) layout; V flipped to (token,
   dim) via PE transposes at f32r rate.
 - the causal/window mask is ADDED ON THE PE (accumulating eye@msk
   matmul into the score psum) so the exp can read psum directly —
   no DVE hop in the score->exp->pv chain; exp(-1e30)=0 keeps masked
   lanes exact. Half-masked tiles run score/mask/exp/pv on the valid
   q-half only.
 - softmax denominators come from 2-moving-column matmuls (probs tile as
   stationary, ones moving): ~8 PE cycles instead of a 512-cycle
   ones-row matmul per tile. Each key tile gets its own psum slot
   (f32r-stationary matmuls silently drop cross-step accumulation) and
   slots are reduced on DVE.
 - o_proj for chunk t is emitted one chunk late so it never waits on a
   just-finished softmax normalization chain; its psum pairs rotate
   through the momentarily-idle score/qkv banks.
Known backend pitfalls encoded here: walrus rejects SB+SB inputs at
different base partitions (rope builds e2 pre-swapped), rejects 1-wide
moving matmul aps, partition_broadcast sources partition 0 only, the
ACT Sin table is unusable for large args (tables DMA'd from host), and
two concurrently-open accumulation groups in one psum tile corrupt.
"""

import numpy as np
import ml_dtypes

B, S, H = 2, 2048, 2048
NH, NKV, HD = 16, 8, 128
WINDOW = 1024
THETA = 100000.0
TP = 4                      # tensor-parallel ways (head groups)
QH = NH // TP               # 4 q heads per core
KVH = NKV // TP             # 2 kv heads per core
NCORES = 8
SCALE = 1.0 / float(np.sqrt(HD))
NEG = -1.0e30

NT = S // 256               # 8 token chunks
NK = H // 128               # 16 contraction tiles

_CACHE = {}
DEBUG = False


def _build_program():
    import concourse.bacc as bacc
    import concourse.mybir as mybir
    import concourse.tile as tile

    f32 = mybir.dt.float32
    f32r = mybir.dt.float32r
    bf16 = mybir.dt.bfloat16
    i32 = mybir.dt.int32
    Exp = mybir.ActivationFunctionType.Exp
    Sin = mybir.ActivationFunctionType.Sin
    mult = mybir.AluOpType.mult
    add = mybir.AluOpType.add

    nc = bacc.Bacc("TRN2", target_bir_lowering=False, debug=False,
                   enable_asserts=False, num_devices=NCORES)

    # host-pre-arranged dram layouts (partition dim first). wpk is grouped
    # by the four col-pairs of the t=0 k-outer passes so each pass can start
    # as soon as its own 1MB group lands instead of waiting for all of wpk.
    hT_d = nc.dram_tensor("hT", [128, NK, S], bf16, kind="ExternalInput")
    wpk_d = nc.dram_tensor("wpk", [128, 4, NK, 256], bf16, kind="ExternalInput")
    wo_d = nc.dram_tensor("wo", [128, QH, H], bf16, kind="ExternalInput")
    cw_d = nc.dram_tensor("cw", [128, 16], f32, kind="ExternalInput")
    cs_d = nc.dram_tensor("cs", [128, S], f32, kind="ExternalInput")
    sn_d = nc.dram_tensor("sn", [128, S], f32, kind="ExternalInput")
    msk_d = nc.dram_tensor("msk", [128, 2048], f32r, kind="ExternalInput")
    eye_d = nc.dram_tensor("eye", [128, 128], f32r, kind="ExternalInput")
    one_d = nc.dram_tensor("one", [128, 8], f32r, kind="ExternalInput")
    yT_d = nc.dram_tensor("yT", [128, NK, S], bf16, kind="ExternalOutput")
    if DEBUG:
        dbg = {
            "dkc": nc.dram_tensor("dkc", [128, 512], f32r, kind="ExternalOutput"),
            "dvt": nc.dram_tensor("dvt", [128, 128], f32r, kind="ExternalOutput"),
            "dqp": nc.dram_tensor("dqp", [128, 512], f32r, kind="ExternalOutput"),
            "da": nc.dram_tensor("da", [128, 256], bf16, kind="ExternalOutput"),
            "drb": nc.dram_tensor("drb", [128, 512], f32r, kind="ExternalOutput"),
            "dpb": nc.dram_tensor("dpb", [128, 512], f32r, kind="ExternalOutput"),
            "dsd": nc.dram_tensor("dsd", [128, 4], f32r, kind="ExternalOutput"),
            "dkb": nc.dram_tensor("dkb", [128, 512], f32, kind="ExternalOutput"),
        }

    from contextlib import ExitStack
    with tile.TileContext(nc) as tc:
        with ExitStack() as stack:
            ent = stack.enter_context
            cp = ent(tc.tile_pool(name="const", bufs=1))
            pp = ent(tc.tile_pool(name="persist", bufs=1))
            wp = ent(tc.tile_pool(name="wts", bufs=1))
            hp = ent(tc.tile_pool(name="ht", bufs=2))
            qpp = ent(tc.tile_pool(name="qp", bufs=3))
            wk = ent(tc.tile_pool(name="work", bufs=3))
            pbp = ent(tc.tile_pool(name="pb", bufs=8))
            atp = ent(tc.tile_pool(name="attn", bufs=2))
            yp = ent(tc.tile_pool(name="yrow", bufs=2))
            ps_qk = ent(tc.tile_pool(name="ps_qk", bufs=2, space="PSUM"))
            ps_scp = ent(tc.tile_pool(name="ps_sc", bufs=2, space="PSUM"))
            ps_pvp = ent(tc.tile_pool(name="ps_pv", bufs=1, space="PSUM"))
            ps_yp = ent(tc.tile_pool(name="ps_y", bufs=1, space="PSUM"))
            ps_dp = ent(tc.tile_pool(name="ps_d", bufs=1, space="PSUM"))
            ps_ms = ent(tc.tile_pool(name="ps_ms", bufs=1, space="PSUM"))
            # ---- constants / tables ----
            cw_sb = cp.tile([128, 16], f32, tag="cw", name="cw")
            eye_sb = cp.tile([128, 128], f32r, tag="eye", name="eye")
            one_sb = cp.tile([128, 8], f32r, tag="one", name="one")
            msk_sb = cp.tile([128, 2048], f32r, tag="msk", name="msk")
            cs_sb = cp.tile([128, S], f32, tag="cs", name="cs")
            sn_sb = cp.tile([128, S], f32, tag="sn", name="sn")

            # weights: wpk in four col-pair groups, wo in two d-halves
            wf = [wp.tile([128, NK * 256], bf16, tag=f"wf{g}", name=f"wf{g}")
                  for g in range(4)]
            wo_sb = [wp.tile([128, 2 * H], bf16, tag=f"wo{h}", name=f"wo{h}")
                     for h in range(2)]

            # persistent k/v state
            kconv = [pp.tile([128, S], f32r, tag=f"kc{i}", name=f"kc{i}")
                     for i in range(KVH)]
            vt = [[pp.tile([128, 128], f32r, tag=f"vt{i}_{j}", name=f"vt{i}_{j}")
                   for j in range(NK)] for i in range(KVH)]
            kbuf = [pp.tile([128, 512], f32, tag=f"kb{i}", name=f"kb{i}")
                    for i in range(KVH)]
            vbuf = [pp.tile([128, 512], f32, tag=f"vb{i}", name=f"vb{i}")
                    for i in range(KVH)]

            # ---- head DMAs, ordered by first use ----
            # head DMAs strictly in consumption order: pass A needs wf0+ht0
            # (k-halved for the earliest possible first matmul), pass B
            # needs wf1, then rope tables/eye/one/msk for chunk-0's
            # process_col + E(0,*), then the remaining weight groups
            ht0 = hp.tile([128, NK * 256], bf16, tag="ht", name="ht0")
            ht0r = ht0[:].rearrange("p (k c) -> p k c", k=NK)
            wr = wf[0][:].rearrange("p (k c) -> p k c", k=NK)
            nc.sync.dma_start(out=cw_sb[:], in_=cw_d[:, :])
            nc.sync.dma_start(out=wr[:, 0:8, :], in_=wpk_d[:, 0, 0:8, :])
            nc.sync.dma_start(out=ht0r[:, 0:8, :], in_=hT_d[:, 0:8, 0:256])
            nc.sync.dma_start(out=wr[:, 8:16, :], in_=wpk_d[:, 0, 8:16, :])
            nc.sync.dma_start(out=ht0r[:, 8:16, :], in_=hT_d[:, 8:16, 0:256])
            nc.sync.dma_start(
                out=wf[1][:].rearrange("p (k c) -> p k c", k=NK),
                in_=wpk_d[:, 1, :, :])
            nc.sync.dma_start(out=cs_sb[:, 0:256], in_=cs_d[:, 0:256])
            nc.sync.dma_start(out=sn_sb[:, 0:256], in_=sn_d[:, 0:256])
            nc.sync.dma_start(out=eye_sb[:], in_=eye_d[:, :])
            nc.sync.dma_start(out=one_sb[:], in_=one_d[:, :])
            nc.sync.dma_start(out=msk_sb[:], in_=msk_d[:, :])
            for g in (2, 3):
                nc.sync.dma_start(
                    out=wf[g][:].rearrange("p (k c) -> p k c", k=NK),
                    in_=wpk_d[:, g, :, :])
            ht1 = hp.tile([128, NK * 256], bf16, tag="ht", name="ht1")
            nc.sync.dma_start(
                out=ht1[:].rearrange("p (k c) -> p k c", k=NK),
                in_=hT_d[:, :, 256:512])
            nc.sync.dma_start(out=cs_sb[:, 256:], in_=cs_d[:, 256:])
            nc.sync.dma_start(out=sn_sb[:, 256:], in_=sn_d[:, 256:])
            nc.sync.dma_start(
                out=wo_sb[0][:].rearrange("p (d c) -> p d c", d=2),
                in_=wo_d[:, 0:2, :])
            nc.sync.dma_start(
                out=wo_sb[1][:].rearrange("p (d c) -> p d c", d=2),
                in_=wo_d[:, 2:4, :])

            hts = [ht0, ht1]

            WF_G = {0: 0, 2: 0, 1: 1, 3: 1, 4: 2, 6: 2, 5: 3, 7: 3}
            WF_S = {0: 0, 2: 1, 1: 0, 3: 1, 4: 0, 6: 1, 5: 0, 7: 1}

            def wf_slice(k, c):
                off = k * 256 + WF_S[c] * 128
                return wf[WF_G[c]][:, off:off + 128]

            # column semantics in host-reordered wpk:
            # 0:k(i=0) 1:q(i=0,h=0) 2:q(i=0,h=1) 3:v(i=0)
            # 4:k(i=1) 5:q(i=1,h=0) 6:q(i=1,h=1) 7:v(i=1)

            def rope(ps, t, dest, doff):
                csl = cs_sb[:, t * 256:(t + 1) * 256]
                snl = sn_sb[:, t * 256:(t + 1) * 256]
                e1 = wk.tile([128, 256], f32, tag="e1", name="e1")
                e2 = wk.tile([128, 256], f32, tag="e2", name="e2")
                nc.vector.tensor_mul(e1[:], ps, csl)
                # e2 is built pre-swapped (x2*s in rows 0:64, x1*s in rows
                # 64:128) so the sub/add below see equal SBUF base
                # partitions (walrus rejects SB+SB inputs at differing bases;
                # the muls are exempt because ps is PSUM)
                nc.vector.tensor_mul(e2[0:64, :], ps[64:128, :],
                                     snl[64:128, :])
                nc.vector.tensor_mul(e2[64:128, :], ps[0:64, :], snl[0:64, :])
                nc.vector.tensor_sub(dest[0:64, doff:doff + 256],
                                     e1[0:64, :], e2[0:64, :])
                nc.vector.tensor_add(dest[64:128, doff:doff + 256],
                                     e2[64:128, :], e1[64:128, :])

            def conv2(t, i, src, w0, w1, dest, doff, cur, prv):
                tmp = wk.tile([128, 256], f32, tag="cvt", name="cvt")
                nc.vector.tensor_scalar_mul(tmp[:], src[:, cur:cur + 256], w1)
                nc.vector.scalar_tensor_tensor(
                    dest[:, doff + 1:doff + 256],
                    src[:, cur:cur + 255], w0, tmp[:, 1:256], mult, add)
                if t == 0:
                    nc.vector.tensor_copy(dest[:, doff:doff + 1], tmp[:, 0:1])
                else:
                    nc.vector.scalar_tensor_tensor(
                        dest[:, doff:doff + 1],
                        src[:, prv + 255:prv + 256], w0, tmp[:, 0:1], mult, add)

            def process_col(t, c, ps, qpair, cur, prv):
                i, kind = c // 4, c % 4
                if kind == 0:          # k head i
                    rope(ps, t, kbuf[i], cur)
                    conv2(t, i, kbuf[i], cw_sb[:, 4 * i:4 * i + 1],
                          cw_sb[:, 4 * i + 1:4 * i + 2],
                          kconv[i], t * 256, cur, prv)
                elif kind == 3:        # v head i
                    nc.scalar.copy(vbuf[i][:, cur:cur + 256], ps)
                    vcb = wk.tile([128, 256], f32r, tag="vcb", name="vcb")
                    conv2(t, i, vbuf[i], cw_sb[:, 4 * i + 2:4 * i + 3],
                          cw_sb[:, 4 * i + 3:4 * i + 4], vcb, 0, cur, prv)
                    for h in range(2):
                        ms = ps_ms.tile([128, 512], f32r, tag="ms", name="ms")
                        nc.tensor.transpose(ms[:, 0:128],
                                            vcb[:, h * 128:(h + 1) * 128],
                                            eye_sb[:])
                        nc.scalar.copy(vt[i][2 * t + h][:], ms[:, 0:128])
                else:                  # q head i, h = kind-1
                    rope(ps, t, qpair[i], (kind - 1) * 256)


            def emit_attn(t, i, qpair, a_lo, a_hi, steal_qk=False):
                qc = t * 256
                jstart = max(0, 2 * t - 8)
                jend = 2 * t + 1
                jorder = list(range(jstart, jend + 1))
                if qc - jstart * 128 == 1024:
                    jorder[0], jorder[1] = jorder[1], jorder[0]
                jfirst, jlast = jorder[0], jorder[-1]

                nj = len(jorder)
                has_edge = qc - jstart * 128 == 1024
                nfull = nj - 1 - (1 if has_edge else 0)
                # denominator psum layout: [p, class-block bm, slot, 2] with
                # bm order (b0, b2, b1, b3). Full tiles take slots 0..nfull-1
                # in both classes; the window-edge tile (valid blocks 0,2)
                # and the newest tile (valid blocks 1,3) share slot nfull.
                BM = {0: 0, 2: 1, 1: 2, 3: 3}
                ps_o = ps_pvp.tile([128, 512], f32, tag="pv", name="pv")
                ps_d = ps_dp.tile([128, 80], f32, tag="pd", name="pd")
                q3 = qpair[:].rearrange("p (h q) -> p h q", h=2)
                full_ctr = 0
                for jn, j in enumerate(jorder):
                    # the last chunk has no qkv work left to hide the
                    # score->exp->pv latency; borrow the idle qkv psum
                    # buffers to deepen the score rotation instead
                    scp = ps_qk if steal_qk and jn % 2 else ps_scp
                    ps_sc = scp.tile([128, 512], f32,
                                     tag="qk" if scp is ps_qk else "sc",
                                     name="sc")
                    delta = qc - j * 128
                    mt = {1024: 0, 896: 1, 0: 2, -128: 3}.get(delta)
                    kcj = kconv[i][:, j * 128:(j + 1) * 128]
                    pb = pbp.tile([128, 512], f32r, tag="pb", name="pb")
                    if (delta == 1024 and j != jfirst) or delta == -128:
                        # half-masked tile: only one q-half of each head is
                        # valid; run score/mask/exp/pv on that half only
                        qh = 0 if delta == 1024 else 1
                        blocks = (0, 2) if qh == 0 else (1, 3)
                        hs = slice(qh * 128, qh * 128 + 128)
                        ps3 = ps_sc[:].rearrange("p (h q) -> p h q", h=2)
                        mk3 = msk_sb[:, mt * 512:(mt + 1) * 512].rearrange(
                            "p (h q) -> p h q", h=2)
                        pb3 = pb[:].rearrange("p (h q) -> p h q", h=2)
                        po3 = ps_o[:].rearrange("p (h q) -> p h q", h=2)
                        nc.tensor.matmul(ps3[:, :, hs], kcj, q3[:, :, hs],
                                         start=True, stop=False)
                        nc.tensor.matmul(ps3[:, :, hs], eye_sb[:],
                                         mk3[:, :, hs],
                                         start=False, stop=True)
                        nc.scalar.activation(pb3[:, :, hs], ps3[:, :, hs],
                                             Exp, bias=0.0, scale=SCALE)
                        nc.tensor.matmul(po3[:, :, hs], vt[i][j][:],
                                         pb3[:, :, hs],
                                         start=False, stop=(j == jlast))
                        slot = nfull
                    else:
                        # full tile; mask added on PE as an accumulating
                        # eye@msk matmul (exp of -1e30 gives exactly 0)
                        blocks = (0, 1, 2, 3)
                        nc.tensor.matmul(ps_sc[:], kcj, qpair[:],
                                         start=True, stop=(mt is None))
                        if delta in (0, 896):
                            # delta=0 masks only the first q-half of each
                            # head (causal diagonal); delta=896 masks only
                            # the second (window edge approaching)
                            mh = slice(0, 128) if delta == 0 else \
                                slice(128, 256)
                            ps3 = ps_sc[:].rearrange("p (h q) -> p h q", h=2)
                            mk3 = msk_sb[:, mt * 512:(mt + 1) * 512].rearrange(
                                "p (h q) -> p h q", h=2)
                            nc.tensor.matmul(ps3[:, :, mh], eye_sb[:],
                                             mk3[:, :, mh],
                                             start=False, stop=True)
                        elif mt is not None:
                            nc.tensor.matmul(ps_sc[:], eye_sb[:],
                                             msk_sb[:, mt * 512:(mt + 1) * 512],
                                             start=False, stop=True)
                        nc.scalar.activation(pb[:], ps_sc[:], Exp,
                                             bias=0.0, scale=SCALE)
                        nc.tensor.matmul(ps_o[:], vt[i][j][:], pb[:],
                                         start=(j == jfirst), stop=(j == jlast))
                        slot = full_ctr
                        full_ctr += 1
                    # softmax denominators: pb as stationary, 2 moving
                    # columns (the ISA rejects 1-wide moving aps). Each j
                    # gets its OWN psum slot — f32r-stationary matmuls
                    # silently drop accumulation across steps — and the
                    # per-j partials are reduced on DVE afterwards.
                    for b in blocks:
                        bm = BM[b]
                        nc.tensor.matmul(
                            ps_d[:, bm * 20 + 2 * slot:bm * 20 + 2 * slot + 2],
                            pb[:, b * 128:(b + 1) * 128],
                            one_sb[:, 0:2], start=True, stop=True)
                # normalize: reduce per-j partials -> 1/sum -> 4 single-
                # column transposes into one partition-0 row -> broadcast
                nA = nfull + (1 if has_edge else 0)
                nB = nfull + 1
                pd4 = ps_d[:].rearrange("p (bm j two) -> p bm j two",
                                        bm=4, two=2)
                red = wk.tile([128, 4], f32, tag="red", name="red")
                nc.vector.tensor_reduce(red[:, 0:2], pd4[:, 0:2, 0:nA, 0:1],
                                        mybir.AxisListType.XY, add)
                nc.vector.tensor_reduce(red[:, 2:4], pd4[:, 2:4, 0:nB, 0:1],
                                        mybir.AxisListType.XY, add)
                sd4 = wk.tile([128, 4], f32r, tag="sd", name="sd")
                with nc.allow_low_precision(reason="f32r is f32 storage"):
                    nc.vector.reciprocal(sd4[:], red[:])
                ms = ps_ms.tile([128, 512], f32r, tag="ms", name="ms")
                for b, bm in BM.items():
                    nc.tensor.transpose(ms[0:1, b * 128:(b + 1) * 128],
                                        sd4[:, bm:bm + 1], eye_sb[:])
                rs = wk.tile([1, 512], f32r, tag="rs", name="rs")
                nc.scalar.copy(rs[:], ms[0:1, :])
                rb = wk.tile([128, 512], f32r, tag="rb", name="rb")
                nc.gpsimd.partition_broadcast(rb[:], rs[:])
                nc.vector.tensor_mul(a_lo[:], ps_o[:, 0:256], rb[:, 0:256])
                nc.vector.tensor_mul(a_hi[:], ps_o[:, 256:512], rb[:, 256:512])
                if DEBUG and t == 0 and i == 0:
                    nc.sync.dma_start(out=dbg["drb"][:, :], in_=rb[:])
                    nc.sync.dma_start(out=dbg["dsd"][:, :], in_=sd4[:])

            def emit_oproj(t, attn, final=False):
                # o_proj partial for chunk t (psum tile per oc pair); lagged
                # one chunk so it never waits on a just-finished softmax
                # normalization. Pairs rotate between the y bank and the
                # momentarily-idle score (and, on the last chunk, qkv)
                # banks for true multi-buffering.
                yrow = yp.tile([128, NK * 256], bf16, tag="yr", name="yr")
                for oc in range(NK):
                    if oc % 2 == 0:
                        sel = (oc // 2) % (3 if final else 2)
                        if sel == 1:
                            yps = ps_scp.tile([128, 512], f32, tag="sc",
                                              name="y")
                        elif sel == 2:
                            yps = ps_qk.tile([128, 512], f32, tag="qk",
                                             name="y")
                        else:
                            yps = ps_yp.tile([128, 512], f32, tag="y", name="y")
                    ps_y = yps[:, (oc % 2) * 256:(oc % 2) * 256 + 256]
                    for d in range(QH):
                        nc.tensor.matmul(
                            ps_y,
                            wo_sb[d // 2][:, (d % 2) * H + oc * 128:
                                          (d % 2) * H + (oc + 1) * 128],
                            attn[d][:], start=(d == 0), stop=(d == QH - 1))
                    if oc % 2 == 0:
                        nc.vector.tensor_copy(yrow[:, oc * 256:(oc + 1) * 256],
                                              ps_y)
                    else:
                        nc.scalar.copy(yrow[:, oc * 256:(oc + 1) * 256],
                                       ps_y)
                    if oc == NK // 2 - 1:
                        nc.sync.dma_start(
                            out=yT_d[:, 0:NK // 2, t * 256:(t + 1) * 256],
                            in_=yrow[:, 0:NK * 128].rearrange(
                                "p (o c) -> p o c", o=NK // 2))
                nc.sync.dma_start(
                    out=yT_d[:, NK // 2:, t * 256:(t + 1) * 256],
                    in_=yrow[:, NK * 128:].rearrange(
                        "p (o c) -> p o c", o=NK // 2))

            pending = []

            # ---- fused chunk loop ----
            for t in range(NT):
                cur, prv = (t % 2) * 256, ((t + 1) % 2) * 256
                if 1 <= t < NT - 1:
                    ht = hp.tile([128, NK * 256], bf16, tag="ht",
                                 name=f"ht{t + 1}")
                    nc.sync.dma_start(
                        out=ht[:].rearrange("p (k c) -> p k c", k=NK),
                        in_=hT_d[:, :, (t + 1) * 256:(t + 2) * 256])
                    hts.append(ht)
                htt = hts[t]
                qpair = [qpp.tile([128, 512], f32r, tag=f"qp{i}",
                                  name=f"qp{i}") for i in range(KVH)]
                attn = [atp.tile([128, 256], bf16, tag=f"a{d}",
                                 name=f"a{d}") for d in range(QH)]
                if t == 0:
                    # k-outer so PE starts as weight DMAs land instead of
                    # waiting for all of wpk. Each pass interleaves two cols
                    # in two DIFFERENT psum tiles: two concurrently-open
                    # accumulation groups inside one tile corrupt results.
                    for half in range(2):
                        pq = [ps_qk.tile([128, 512], f32, tag="qk", name="qk")
                              for _ in range(2)]
                        for sub in range(2):
                            cols = (4 * half + sub, 4 * half + sub + 2)
                            for k in range(NK):
                                for ci, c in enumerate(cols):
                                    nc.tensor.matmul(
                                        pq[ci][:, sub * 256:sub * 256 + 256],
                                        wf_slice(k, c),
                                        htt[:, k * 256:(k + 1) * 256],
                                        start=(k == 0), stop=(k == NK - 1))
                        for c in range(4):
                            # col 4h+0 -> pq[0] reg0, 4h+1 -> pq[0] reg1,
                            # col 4h+2 -> pq[1] reg0, 4h+3 -> pq[1] reg1
                            ti = (0, 0, 1, 1)[c]
                            rg = (0, 1, 0, 1)[c] * 256
                            process_col(t, 4 * half + c,
                                        pq[ti][:, rg:rg + 256],
                                        qpair, cur, prv)
                            if 4 * half + c == 3:
                                emit_attn(t, 0, qpair[0], attn[0], attn[1])
                    emit_attn(t, 1, qpair[1], attn[2], attn[3])
                else:
                    for cp2 in range(4):
                        pq = ps_qk.tile([128, 512], f32, tag="qk", name="qk")
                        for half in range(2):
                            # process each col right after its own matmuls:
                            # its rope overlaps the sibling col's matmuls
                            # and the psum region frees a pair earlier
                            c = 2 * cp2 + half
                            ps = pq[:, half * 256:half * 256 + 256]
                            for k in range(NK):
                                nc.tensor.matmul(
                                    ps, wf_slice(k, c),
                                    htt[:, k * 256:(k + 1) * 256],
                                    start=(k == 0), stop=(k == NK - 1))
                            process_col(t, c, ps, qpair, cur, prv)
                            if c == 3:
                                emit_attn(t, 0, qpair[0], attn[0], attn[1])
                        if cp2 == 1 and pending:
                            emit_oproj(*pending.pop())
                    emit_attn(t, 1, qpair[1], attn[2], attn[3],
                              steal_qk=(t == NT - 1))
                if DEBUG and t == 0:
                    nc.sync.dma_start(out=dbg["dkc"][:, :],
                                      in_=kconv[0][:, 0:512])
                    nc.sync.dma_start(out=dbg["dkb"][:, :], in_=kbuf[0][:])
                    nc.sync.dma_start(out=dbg["dvt"][:, :], in_=vt[0][0][:])
                    nc.sync.dma_start(out=dbg["dqp"][:, :], in_=qpair[0][:])
                    nc.sync.dma_start(out=dbg["da"][:, :], in_=attn[0][:])
                pending.append((t, attn))
            emit_oproj(*pending.pop(), final=True)

    nc.finalize()
    return nc


def _host_inputs(hidden, W_pack, W_o, conv_k, conv_v):
    """Per-core input maps (host-side sharding + dram pre-arrangement)."""
    bfloat16 = ml_dtypes.bfloat16

    kk = np.arange(128)[:, None]
    qq = np.arange(256)[None, :]

    def double(m):
        return np.concatenate([m, m], axis=1).astype(np.float32)

    t0 = double(np.where(kk <= qq, 0.0, NEG))             # delta = 0
    tm128 = double(np.where(kk <= qq - 128, 0.0, NEG))    # delta = -128
    w896 = double(np.where(qq - kk < 128, 0.0, NEG))      # delta = 896
    w1024 = double(np.where(qq < kk, 0.0, NEG))           # delta = 1024
    msk = np.concatenate([w1024, w896, t0, tm128], axis=1)  # (128, 2048)

    eye = np.eye(128, dtype=np.float32)
    one = np.ones((128, 8), dtype=np.float32)

    inv_freq = (1.0 / (THETA ** (np.arange(0, HD, 2, dtype=np.float64) / HD))
                ).astype(np.float32)
    freqs = np.outer(np.arange(S, dtype=np.float64), inv_freq.astype(np.float64))
    cos = np.cos(freqs).T.astype(np.float32)             # (64, S)
    sin = np.sin(freqs).T.astype(np.float32)
    cs = np.ascontiguousarray(np.concatenate([cos, cos], axis=0))  # (128, S)
    sn = np.ascontiguousarray(np.concatenate([sin, sin], axis=0))

    in_maps = []
    for c in range(NCORES):
        b, g = c // TP, c % TP
        # hT: [128, NK, S] with hT[p, k, s] = hidden[b, s, k*128+p]
        hT = np.ascontiguousarray(
            hidden[b].T.reshape(NK, 128, S).transpose(1, 0, 2)
        ).astype(bfloat16)
        # wpk columns per 128-group: [k0, q0, q1, v0, k1, q2, q3, v1]
        cols = []
        for i in range(KVH):
            cols.append(W_pack[:, NH * HD + (2 * g + i) * 128:
                               NH * HD + (2 * g + i + 1) * 128])      # k_i
            cols.append(W_pack[:, (4 * g + 2 * i) * 128:
                               (4 * g + 2 * i + 1) * 128])            # q_{2i}
            cols.append(W_pack[:, (4 * g + 2 * i + 1) * 128:
                               (4 * g + 2 * i + 2) * 128])            # q_{2i+1}
            cols.append(W_pack[:, NH * HD + NKV * HD + (2 * g + i) * 128:
                               NH * HD + NKV * HD + (2 * g + i + 1) * 128])
        wcat = np.concatenate(cols, axis=1)              # (2048, 1024)
        # group columns by the t=0 pass pairs (0,2),(1,3),(4,6),(5,7):
        # wpk[p, g, k, sub*128+cc] = wcat[k*128+p, pair[g][sub]*128+cc]
        w4 = wcat.reshape(NK, 128, 8, 128)[:, :, [0, 2, 1, 3, 4, 6, 5, 7], :]
        wpk = np.ascontiguousarray(
            w4.reshape(NK, 128, 4, 256).transpose(1, 2, 0, 3)).astype(bfloat16)
        # wo: [128, QH, H] with wo[p, d, o] = W_o[g*512 + d*128 + p, o]
        wo = np.ascontiguousarray(
            W_o[g * 512:(g + 1) * 512, :].reshape(QH, 128, H)
            .transpose(1, 0, 2)).astype(bfloat16)
        cwv = np.zeros(16, np.float32)
        for i in range(KVH):
            cwv[4 * i] = conv_k[2 * g + i, 0]
            cwv[4 * i + 1] = conv_k[2 * g + i, 1]
            cwv[4 * i + 2] = conv_v[2 * g + i, 0]
            cwv[4 * i + 3] = conv_v[2 * g + i, 1]
        cw = np.broadcast_to(cwv, (128, 16)).copy()
        cw[:, 8] = inv_freq[np.arange(128) % 64]          # rope inv-freq
        cw[:, 9] = np.pi / 2                              # Sin->Cos bias
        in_maps.append({
            "hT": hT, "wpk": wpk, "wo": wo, "cw": cw, "cs": cs, "sn": sn,
            "msk": msk, "eye": eye, "one": one,
        })
    return in_maps


def run_cores(in_maps, trace=False, **kw):
    from concourse.bass_utils import run_bass_kernel_spmd
    if "nc" not in _CACHE:
        _CACHE["nc"] = _build_program()
    return run_bass_kernel_spmd(_CACHE["nc"], in_maps, list(range(NCORES)),
                                trace=trace, **kw)


def kernel(hidden, W_pack, W_o, conv_k, conv_v):
    hidden = np.asarray(hidden, np.float32)
    W_pack = np.asarray(W_pack, np.float32)
    W_o = np.asarray(W_o, np.float32)
    conv_k = np.asarray(conv_k, np.float32)
    conv_v = np.asarray(conv_v, np.float32)
    in_maps = _host_inputs(hidden, W_pack, W_o, conv_k, conv_v)
    res = run_cores(in_maps)
    out = np.zeros((B, S, H), np.float32)
    for c in range(NCORES):
        b = c // TP
        y = np.asarray(res.results[c]["yT"]).astype(np.float32)  # [128,NK,S]
        out[b] += y.transpose(2, 1, 0).reshape(S, H)
    return out
